# revision 9
# baseline (speedup 1.0000x reference)
"""Pauli-Y gate on qubit 5 of a 22-qubit state, batch 8 (raw Bass).

Math: state viewed as [B, 32a, 2j, 65536c] complex64.
  y[a,0,c] = -i * x[a,1,c]  ->  re = +im_src, im = -re_src   (src j=1)
  y[a,1,c] = +i * x[a,0,c]  ->  re = -im_src, im = +re_src   (src j=0)

Pure data movement: per core (1 batch row) 32MB in, 32MB out. The only
compute is sign flips and the re/im interleave into complex64 layout,
done on ACT with stride-2 free-dim writes in SBUF so all DMA transfers
stay large and contiguous.

Engine programs (raw Bass, no Tile):
  SP  (sync):   in-DMAs (HWDGE), WAR-gated on compute progress
  ACT (scalar): interleave compute (ACTIVATE copy/neg) + out-DMAs (HWDGE)

All cross-engine waits are standalone sequencer waits — the HWDGE
direct-2D DMA lowering only supports a single attached wait, so DMA
instructions carry none.

DMA completion counting uses one semaphore PER BUFFER SLOT: completion
increments of different DMAs on one ring can interleave (each of the 16
SDMA engines increments independently), so a single cumulative counter
can be satisfied by increments of *later* DMAs while an earlier one is
still landing. Per-slot semaphores are exact because the pipeline gating
guarantees only one iteration's DMAs touch a slot semaphore at a time.

Sharding: data-parallel over batch, one row per NeuronCore (8 rows, 8
cores).
"""

from contextlib import ExitStack

import numpy as np

import concourse.bass as bass
import concourse.mybir as mybir
from concourse.bass_utils import run_bass_kernel_spmd

B = 8
A, J, P, F = 32, 2, 128, 512  # D = A*J*P*F = 4194304
D = A * J * P * F
G = 4  # a-blocks per iteration
NIT = (A // G) * J  # 16 iterations
NBUF = 4  # buffered iteration sets in SBUF

_nc_cache = None


def _build():
    global _nc_cache
    if _nc_cache is not None:
        return _nc_cache

    nc = bass.Bass()
    re = nc.dram_tensor("re", [D], mybir.dt.float32, kind="ExternalInput")
    im = nc.dram_tensor("im", [D], mybir.dt.float32, kind="ExternalInput")
    out = nc.dram_tensor("out", [2 * D], mybir.dt.float32, kind="ExternalOutput")

    re_v = re.rearrange("(a j p f) -> a j p f", a=A, j=J, p=P, f=F)
    im_v = im.rearrange("(a j p f) -> a j p f", a=A, j=J, p=P, f=F)
    out_v = out.rearrange("(a j p f) -> a j p f", a=A, j=J, p=P, f=2 * F)

    f32 = mybir.dt.float32
    iters = [(j, g) for j in range(J) for g in range(A // G)]

    with ExitStack() as ctx:
        re_b = ctx.enter_context(nc.sbuf_tensor([P, NBUF * G * F], f32))
        im_b = ctx.enter_context(nc.sbuf_tensor([P, NBUF * G * F], f32))
        out_b = ctx.enter_context(nc.sbuf_tensor([P, NBUF * G * 2 * F], f32))
        s_in = [
            ctx.enter_context(nc.semaphore(f"s_in{k}")) for k in range(NBUF)
        ]
        s_out = [
            ctx.enter_context(nc.semaphore(f"s_out{k}")) for k in range(NBUF)
        ]
        s_cmp = ctx.enter_context(nc.semaphore("s_cmp"))
        block = ctx.enter_context(nc.Block())

        def in_slot(s):
            return slice(s * G * F, (s + 1) * G * F)

        def out_slot(s):
            return slice(s * G * 2 * F, (s + 1) * G * 2 * F)

        @block.sync
        def _(sync):
            for n, (j, g) in enumerate(iters):
                s = n % NBUF
                sj = 1 - j
                a0, a1 = g * G, (g + 1) * G
                if n >= NBUF:
                    # compute of iter n-NBUF must have read the in tiles
                    sync.wait_ge(s_cmp, 2 * (n - NBUF + 1))
                sync.dma_start(
                    out=re_b[:, in_slot(s)].rearrange("p (a f) -> p a f", a=G),
                    in_=re_v[a0:a1, sj].transpose([1, 0, 2]),
                ).then_inc(s_in[s], 16)
                sync.dma_start(
                    out=im_b[:, in_slot(s)].rearrange("p (a f) -> p a f", a=G),
                    in_=im_v[a0:a1, sj].transpose([1, 0, 2]),
                ).then_inc(s_in[s], 16)

        @block.scalar
        def _(scalar):
            for n, (j, g) in enumerate(iters):
                s = n % NBUF
                cyc = n // NBUF
                a0, a1 = g * G, (g + 1) * G
                if n >= NBUF:
                    # out-DMA of iter n-NBUF must have drained the out tile
                    scalar.wait_ge(s_out[s], 16 * cyc)
                scalar.wait_ge(s_in[s], 32 * (cyc + 1))
                ot = out_b[:, out_slot(s)]
                ev = ot[:, 0::2]
                od = ot[:, 1::2]
                rt = re_b[:, in_slot(s)]
                it_ = im_b[:, in_slot(s)]
                if j == 0:
                    scalar.copy(ev, it_).then_inc(s_cmp, 1)  # re_out = +im_src
                    scalar.mul(od, rt, -1.0).then_inc(s_cmp, 1)  # im_out = -re
                else:
                    scalar.mul(ev, it_, -1.0).then_inc(s_cmp, 1)  # re_out = -im
                    scalar.copy(od, rt).then_inc(s_cmp, 1)  # im_out = +re_src
                # engine pipelines are deep: the sequencer dispatches the
                # out-DMA before the ACTIVATEs complete unless we wait.
                scalar.wait_ge(s_cmp, 2 * (n + 1))
                scalar.dma_start(
                    out=out_v[a0:a1, j].transpose([1, 0, 2]),
                    in_=ot.rearrange("p (a f) -> p a f", a=G),
                ).then_inc(s_out[s], 16)
            for k in range(NBUF):
                scalar.wait_ge(s_out[k], 16 * (NIT // NBUF))

    _nc_cache = nc
    return nc


def kernel(state_re: np.ndarray, state_im: np.ndarray) -> np.ndarray:
    state_re = np.ascontiguousarray(np.asarray(state_re, dtype=np.float32))
    state_im = np.ascontiguousarray(np.asarray(state_im, dtype=np.float32))
    assert state_re.shape == (B, D) and state_im.shape == (B, D)

    nc = _build()
    in_maps = [{"re": state_re[b], "im": state_im[b]} for b in range(B)]
    res = run_bass_kernel_spmd(nc, in_maps, core_ids=list(range(B)))
    rows = [res.results[b]["out"].view(np.complex64) for b in range(B)]
    return np.stack(rows, axis=0)


# revision 25
# speedup vs baseline: 1.1131x; 1.1131x over previous
"""Pauli-Y gate on qubit 5 of a 22-qubit state, batch 8 — TRN2 Bass kernel.

Math: state viewed as [B, 32a, 2j, 65536c] complex64 (qubit 5 is the j
axis; a = qubits 0-4, c = qubits 6-21 in the reference's ordering).
  y[a,0,c] = -i * x[a,1,c]  ->  re = +im_src, im = -re_src   (src j=1)
  y[a,1,c] = +i * x[a,0,c]  ->  re = -im_src, im = +re_src   (src j=0)

Pure data movement: per core (1 batch row) 32MB in, 32MB out. The only
compute is sign flips and the re/im interleave into complex64 layout,
done on ACT with stride-2 free-dim writes in SBUF so every DMA transfer
stays large and contiguous (2KB/4KB runs per partition).

Engine programs (raw Bass, no Tile):
  SP  (sync):   in-DMAs (HWDGE ring 1), WAR-gated on compute progress
  ACT (scalar): interleave compute (ACTIVATE copy/neg) + out-DMAs
                (HWDGE ring 2)

Three synchronization rules this kernel is built around (all verified
the hard way — CoreSim's race detector catches each):
  1. The HWDGE direct-2D DMA lowering supports a single attached sync
     wait, so DMA instructions carry none; all waits are standalone
     sequencer `wait_ge` instructions.
  2. Sequencers do NOT wait for instruction completion before
     dispatching the next instruction (deep pipelines), so even
     same-engine ACTIVATE -> out-DMA needs a semaphore round trip.
  3. DMA-completion increments of different DMAs on one ring interleave
     (each of the 16 SDMA engines increments independently), so a
     cumulative completion counter can be satisfied by increments of
     *later* DMAs while an earlier one is still landing. Completion
     counting therefore uses one semaphore PER BUFFER SLOT; pipeline
     gating guarantees only one iteration's DMAs touch a slot
     semaphore at a time, which makes the counts exact.

Pipelining: G=2 a-blocks per iteration (512KB per in-DMA, 1MB per
out-DMA), NBUF=8 buffered iteration sets (128KB/partition of SBUF).
Measured on trn2: ~178us/core typical (~410 GB/s sustained aggregate
DMA, vs ~179us naive roofline at 358 GB/s); coarser or finer tilings
and 3-ring/DVE-split variants measured slower.

Sharding: data-parallel over batch, one row per NeuronCore (8 rows, 8
cores). Full inputs in, full output out; complex64 assembled on host by
viewing the interleaved f32 pairs.
"""

from contextlib import ExitStack

import numpy as np

import concourse.bass as bass
import concourse.mybir as mybir
from concourse.bass_utils import run_bass_kernel_spmd

B = 8
A, J, P, F = 32, 2, 128, 512  # D = A*J*P*F = 4194304
D = A * J * P * F
G = 2  # a-blocks per iteration
NIT = (A // G) * J  # 32 iterations
NBUF = 8  # buffered iteration sets in SBUF

_nc_cache = None


def _build():
    global _nc_cache
    if _nc_cache is not None:
        return _nc_cache

    nc = bass.Bass()
    re = nc.dram_tensor("re", [D], mybir.dt.float32, kind="ExternalInput")
    im = nc.dram_tensor("im", [D], mybir.dt.float32, kind="ExternalInput")
    out = nc.dram_tensor("out", [2 * D], mybir.dt.float32, kind="ExternalOutput")

    re_v = re.rearrange("(a j p f) -> a j p f", a=A, j=J, p=P, f=F)
    im_v = im.rearrange("(a j p f) -> a j p f", a=A, j=J, p=P, f=F)
    out_v = out.rearrange("(a j p f) -> a j p f", a=A, j=J, p=P, f=2 * F)

    f32 = mybir.dt.float32
    iters = [(j, g * G) for j in range(J) for g in range(A // G)]

    with ExitStack() as ctx:
        re_b = ctx.enter_context(nc.sbuf_tensor([P, NBUF * G * F], f32))
        im_b = ctx.enter_context(nc.sbuf_tensor([P, NBUF * G * F], f32))
        out_b = ctx.enter_context(nc.sbuf_tensor([P, NBUF * G * 2 * F], f32))
        s_in = [
            ctx.enter_context(nc.semaphore(f"s_in{k}")) for k in range(NBUF)
        ]
        s_out = [
            ctx.enter_context(nc.semaphore(f"s_out{k}")) for k in range(NBUF)
        ]
        s_cmp = ctx.enter_context(nc.semaphore("s_cmp"))
        block = ctx.enter_context(nc.Block())

        def in_slot(s):
            return slice(s * G * F, (s + 1) * G * F)

        def out_slot(s):
            return slice(s * G * 2 * F, (s + 1) * G * 2 * F)

        @block.sync
        def _(sync):
            for n, (j, a0) in enumerate(iters):
                s = n % NBUF
                sj = 1 - j
                a1 = a0 + G
                if n >= NBUF:
                    # compute of iter n-NBUF must have read the in tiles
                    sync.wait_ge(s_cmp, 2 * (n - NBUF + 1))
                sync.dma_start(
                    out=re_b[:, in_slot(s)].rearrange("p (a f) -> p a f", a=G),
                    in_=re_v[a0:a1, sj].transpose([1, 0, 2]),
                ).then_inc(s_in[s], 16)
                sync.dma_start(
                    out=im_b[:, in_slot(s)].rearrange("p (a f) -> p a f", a=G),
                    in_=im_v[a0:a1, sj].transpose([1, 0, 2]),
                ).then_inc(s_in[s], 16)

        @block.scalar
        def _(scalar):
            for n, (j, a0) in enumerate(iters):
                s = n % NBUF
                cyc = n // NBUF
                a1 = a0 + G
                if n >= NBUF:
                    # out-DMA of iter n-NBUF must have drained the out tile
                    scalar.wait_ge(s_out[s], 16 * cyc)
                scalar.wait_ge(s_in[s], 32 * (cyc + 1))
                ot = out_b[:, out_slot(s)]
                ev = ot[:, 0::2]
                od = ot[:, 1::2]
                rt = re_b[:, in_slot(s)]
                it_ = im_b[:, in_slot(s)]
                if j == 0:
                    scalar.copy(ev, it_).then_inc(s_cmp, 1)  # re = +im_src
                    scalar.mul(od, rt, -1.0).then_inc(s_cmp, 1)  # im = -re_src
                else:
                    scalar.mul(ev, it_, -1.0).then_inc(s_cmp, 1)  # re = -im
                    scalar.copy(od, rt).then_inc(s_cmp, 1)  # im = +re_src
                # engine pipelines are deep: the sequencer would dispatch the
                # out-DMA before the ACTIVATEs complete unless we wait.
                scalar.wait_ge(s_cmp, 2 * (n + 1))
                scalar.dma_start(
                    out=out_v[a0:a1, j].transpose([1, 0, 2]),
                    in_=ot.rearrange("p (a f) -> p a f", a=G),
                ).then_inc(s_out[s], 16)
            for k in range(NBUF):
                scalar.wait_ge(s_out[k], 16 * (NIT // NBUF))

    _nc_cache = nc
    return nc


def kernel(state_re: np.ndarray, state_im: np.ndarray) -> np.ndarray:
    state_re = np.ascontiguousarray(np.asarray(state_re, dtype=np.float32))
    state_im = np.ascontiguousarray(np.asarray(state_im, dtype=np.float32))
    assert state_re.shape == (B, D) and state_im.shape == (B, D)

    nc = _build()
    in_maps = [{"re": state_re[b], "im": state_im[b]} for b in range(B)]
    res = run_bass_kernel_spmd(nc, in_maps, core_ids=list(range(B)))
    rows = [res.results[b]["out"].view(np.complex64) for b in range(B)]
    return np.stack(rows, axis=0)
